# revision 2
# baseline (speedup 1.0000x reference)
"""BiLSTM-CRF loss kernel for Trainium2, 8 NeuronCores, data-parallel over batch.

Contract: kernel(**inputs) takes the FULL inputs from reference.setup_inputs()
and returns the FULL scalar output (mean CRF NLL). Internally each of the 8
cores processes 8 batch elements end-to-end (embedding gather, BiLSTM,
emissions, gold score, CRF forward in exp space); the host averages the 64
per-batch partials.
"""
import os
import sys
from contextlib import ExitStack

import numpy as np

for _p in ("/opt/trn_rl_repo", "/root/.axon_site/_ro/trn_rl_repo"):
    if os.path.isdir(_p) and _p not in sys.path:
        sys.path.insert(0, _p)

import concourse.bass as bass
import concourse.tile as tile
from concourse import bacc, mybir
from concourse.bass_utils import run_bass_kernel_spmd
from concourse.masks import make_identity

F32 = mybir.dt.float32
BF16 = mybir.dt.bfloat16
I32 = mybir.dt.int32
AF = mybir.ActivationFunctionType
ALU = mybir.AluOpType

# Problem shapes (hardcoded per harness contract).
V, E, H, HID, TT = 50257, 512, 512, 1024, 64
B, S = 64, 512
NCORES = 8
BC = B // NCORES          # batch per core
G4 = 4 * H                # gate width
TOK = S * BC              # tokens per core, ordered (t, b)
NEG = -10000.0
C_TRANS = 4.0             # CRF transition exp shift
RENORM = 6                # CRF renormalization period


def build_program(nsteps=S):
    """Build the per-core Bass program. nsteps<S shrinks the time dim for sim."""
    Sq = nsteps
    TOKq = Sq * BC
    nc = bacc.Bacc("TRN2", target_bir_lowering=False, debug=False,
                   num_devices=NCORES)

    din = lambda name, shp, dt=F32: nc.dram_tensor(name, shp, dt, kind="ExternalInput").ap()
    tok_idx = din("tok_idx", [TOKq, 1], I32)
    tags_next = din("tags_next", [TOKq, 1])
    tags_prev = din("tags_prev", [TOKq, 1])
    emb = din("emb", [V, E])
    wihT = {d: din(f"wihT_{d}", [E, G4]) for d in "fb"}
    whhT = {d: din(f"whhT_{d}", [H, G4]) for d in "fb"}
    b_ih = {d: din(f"b_ih_{d}", [G4, 1]) for d in "fb"}
    b_hh = {d: din(f"b_hh_{d}", [G4, 1]) for d in "fb"}
    woutT = din("woutT", [HID, TT])
    b_out = din("b_out", [TT, 1])
    transT = din("transT", [TT, TT])
    out_dram = nc.dram_tensor("out", [1, BC], F32, kind="ExternalOutput").ap()

    pre = {d: nc.dram_tensor(f"pre_{d}", [TOKq, G4], BF16).ap() for d in "fb"}

    NTILE = TOKq // 128       # 128-token tiles
    NCH = max(TOKq // 512, 1)  # 512-token chunks
    CHW = min(TOKq, 512)      # chunk width in tokens

    with tile.TileContext(nc) as tc, ExitStack() as es:
        cpool = es.enter_context(tc.tile_pool(name="const", bufs=1))
        pp = es.enter_context(tc.tile_pool(name="persist", bufs=1))

        identf = cpool.tile([128, 128], F32)
        make_identity(nc, identf[:])
        identb = cpool.tile([128, 128], BF16)
        nc.vector.tensor_copy(identb[:], identf[:])
        ones_col = cpool.tile([TT, 1], F32)
        nc.vector.memset(ones_col[:], 1.0)
        ones_row = cpool.tile([1, 128], F32)
        nc.vector.memset(ones_row[:], 1.0)
        iota_i = cpool.tile([TT, 1], I32)
        nc.gpsimd.iota(iota_i[:], pattern=[[0, 1]], base=0, channel_multiplier=1)
        iota_f = cpool.tile([TT, 1], F32)
        nc.vector.tensor_copy(iota_f[:], iota_i[:])

        # whole-kernel persistents: recurrent weights, transitions, out bias
        whh_sb = {d: [pp.tile([128, G4], BF16, tag=f"whh{d}{k}", name=f"whh{d}{k}")
                      for k in range(4)] for d in "fb"}
        trans_sb = pp.tile([TT, TT], F32, tag="trans_sb")
        nc.sync.dma_start(trans_sb[:], transT[:])
        bout_sb = pp.tile([TT, 1], F32, tag="bout_sb")
        nc.sync.dma_start(bout_sb[:], b_out[:])

        # ======== Phases A+B (xT scope): gather, weights, input projection ====
        with tc.tile_pool(name="ppx", bufs=1) as ppx:
            xT = [ppx.tile([128, TOKq], BF16, tag=f"xT{k}", name=f"xT{k}")
                  for k in range(4)]

            with tc.tile_pool(name="gather", bufs=4) as gp, \
                 tc.tile_pool(name="pgather", bufs=4, space="PSUM") as pg:
                for i in range(NTILE):
                    idx_t = gp.tile([128, 1], I32, tag="idx")
                    nc.sync.dma_start(idx_t[:], tok_idx[i * 128:(i + 1) * 128, :])
                    xg = gp.tile([128, E], F32, tag="xg")
                    nc.gpsimd.indirect_dma_start(
                        out=xg[:], out_offset=None, in_=emb[:],
                        in_offset=bass.IndirectOffsetOnAxis(ap=idx_t[:, :1], axis=0))
                    xgb = gp.tile([128, E], BF16, tag="xgb")
                    nc.vector.tensor_copy(xgb[:], xg[:])
                    for k in range(4):
                        tp = pg.tile([128, 128], BF16, tag="tp")
                        nc.tensor.transpose(out=tp[:], in_=xgb[:, k * 128:(k + 1) * 128],
                                            identity=identb[:])
                        nc.vector.tensor_copy(xT[k][:, i * 128:(i + 1) * 128], tp[:])

            with tc.tile_pool(name="wload", bufs=2) as wl:
                for d in "fb":
                    for k in range(4):
                        wf = wl.tile([128, G4], F32, tag="wf")
                        nc.sync.dma_start(wf[:], whhT[d][k * 128:(k + 1) * 128, :])
                        nc.vector.tensor_copy(whh_sb[d][k][:], wf[:])

            with tc.tile_pool(name="iproj", bufs=3) as ip, \
                 tc.tile_pool(name="ipw", bufs=1) as ipw, \
                 tc.tile_pool(name="pproj", bufs=3, space="PSUM") as ppj:
                for d in "fb":
                    wih_sb = []
                    for k in range(4):
                        wb = ipw.tile([128, G4], BF16, tag=f"wih{d}{k}", name=f"wih{d}{k}")
                        for hh in range(2):
                            wf2 = ip.tile([128, G4 // 2], F32, tag="wf2")
                            nc.sync.dma_start(
                                wf2[:], wihT[d][k * 128:(k + 1) * 128,
                                                hh * (G4 // 2):(hh + 1) * (G4 // 2)])
                            nc.vector.tensor_copy(
                                wb[:, hh * (G4 // 2):(hh + 1) * (G4 // 2)], wf2[:])
                        wih_sb.append(wb)
                    bias_bc = []
                    for ch in range(4):
                        bi = ip.tile([1, 512], F32, tag="bi")
                        nc.sync.dma_start(bi[:], b_ih[d][ch * 512:(ch + 1) * 512, :].rearrange("g 1 -> 1 g"))
                        bh = ip.tile([1, 512], F32, tag="bh")
                        nc.sync.dma_start(bh[:], b_hh[d][ch * 512:(ch + 1) * 512, :].rearrange("g 1 -> 1 g"))
                        nc.vector.tensor_add(bi[:], bi[:], bh[:])
                        bps = ppj.tile([128, 512], F32, tag="bps")
                        nc.tensor.matmul(bps[:], ones_row[:1, :128], bi[:], start=True, stop=True)
                        bb = ipw.tile([128, 512], BF16, tag=f"bb{d}{ch}", name=f"bb{d}{ch}")
                        nc.vector.tensor_copy(bb[:], bps[:])
                        bias_bc.append(bb)
                    for i in range(NTILE):
                        for ch in range(4):
                            acc = ppj.tile([128, 512], F32, tag="acc")
                            for k in range(4):
                                nc.tensor.matmul(
                                    acc[:], xT[k][:, i * 128:(i + 1) * 128],
                                    wih_sb[k][:, ch * 512:(ch + 1) * 512],
                                    start=(k == 0), stop=(k == 3))
                            pre_t = ip.tile([128, 512], BF16, tag="pre_t")
                            nc.vector.tensor_add(pre_t[:], acc[:], bias_bc[ch][:])
                            nc.sync.dma_start(
                                pre[d][i * 128:(i + 1) * 128, ch * 512:(ch + 1) * 512],
                                pre_t[:])

        # ======== Phase C..F scope: h history + emissions =====================
        pph = es.enter_context(tc.tile_pool(name="pph", bufs=1))
        hhist = {d: pph.tile([128, 4 * TOKq], BF16, tag=f"hhist{d}", name=f"hhist{d}")
                 for d in "fb"}
        em_sb = pph.tile([TT, TOKq], BF16, tag="em_sb")

        # ---- Phase C: BiLSTM recurrence, directions interleaved ----
        with tc.tile_pool(name="rec", bufs=3) as rp, \
             tc.tile_pool(name="recst", bufs=1) as rs, \
             tc.tile_pool(name="prebuf", bufs=4) as pb, \
             tc.tile_pool(name="pgate", bufs=4, space="PSUM") as pgate, \
             tc.tile_pool(name="ptr", bufs=2, space="PSUM") as ptr:
            st = {}
            for d in "fb":
                c_sb = rs.tile([BC, H], F32, tag=f"c{d}", name=f"c{d}")
                nc.vector.memset(c_sb[:], 0.0)
                st[d] = c_sb

            def lstm_step(d, t, first):
                c_sb = st[d]
                pre_row = pb.tile([BC, G4], BF16, tag=f"prerow{d}")
                nc.sync.dma_start(pre_row[:], pre[d][t * BC:(t + 1) * BC, :])
                tprev = (t + 1) if d == "b" else (t - 1)
                gsb = []
                for ch in range(4):
                    func = AF.Tanh if ch == 2 else AF.Sigmoid
                    ga = rp.tile([BC, 512], F32, tag=f"ga{ch}", name=f"ga{d}{t}_{ch}")
                    if first:
                        nc.scalar.activation(ga[:], pre_row[:, ch * 512:(ch + 1) * 512], func)
                    else:
                        acc = pgate.tile([BC, 512], F32, tag="gacc", name=f"gacc{d}{t}_{ch}")
                        for k in range(4):
                            nc.tensor.matmul(
                                acc[:],
                                hhist[d][:, (k * TOKq + tprev * BC):(k * TOKq + (tprev + 1) * BC)],
                                whh_sb[d][k][:, ch * 512:(ch + 1) * 512],
                                start=(k == 0), stop=(k == 3))
                        av = rp.tile([BC, 512], F32, tag=f"av{ch}", name=f"av{d}{t}_{ch}")
                        nc.vector.tensor_add(av[:], acc[:], pre_row[:, ch * 512:(ch + 1) * 512])
                        nc.scalar.activation(ga[:], av[:], func)
                    gsb.append(ga)
                sig_i, sig_f, tan_g, sig_o = gsb
                nc.vector.tensor_mul(c_sb[:], sig_f[:], c_sb[:])
                tmp = rp.tile([BC, H], F32, tag="tmp", name=f"tmp{d}{t}")
                nc.vector.tensor_mul(tmp[:], sig_i[:], tan_g[:])
                nc.vector.tensor_add(c_sb[:], c_sb[:], tmp[:])
                tan_c = rp.tile([BC, H], F32, tag="tanc", name=f"tanc{d}{t}")
                nc.scalar.activation(tan_c[:], c_sb[:], AF.Tanh)
                h_sb = rp.tile([BC, H], F32, tag="hsb", name=f"hsb{d}{t}")
                nc.vector.tensor_mul(h_sb[:], sig_o[:], tan_c[:])
                for k in range(4):
                    tp = ptr.tile([128, BC], F32, tag="htp", name=f"htp{d}{t}_{k}")
                    nc.tensor.transpose(out=tp[:], in_=h_sb[:, k * 128:(k + 1) * 128],
                                        identity=identf[:BC, :BC])
                    nc.vector.tensor_copy(
                        hhist[d][:, (k * TOKq + t * BC):(k * TOKq + (t + 1) * BC)], tp[:])

            for j in range(Sq):
                lstm_step("f", j, first=(j == 0))
                lstm_step("b", Sq - 1 - j, first=(j == 0))

        # ---- Phase D: emissions ----
        with tc.tile_pool(name="emp", bufs=2) as emp, \
             tc.tile_pool(name="empw", bufs=1) as empw, \
             tc.tile_pool(name="pem", bufs=2, space="PSUM") as pem:
            wout_sb = []
            for k in range(8):
                wf3 = emp.tile([128, TT], F32, tag="wf3")
                nc.sync.dma_start(wf3[:], woutT[k * 128:(k + 1) * 128, :])
                wb3 = empw.tile([128, TT], BF16, tag=f"wout{k}", name=f"wout{k}")
                nc.vector.tensor_copy(wb3[:], wf3[:])
                wout_sb.append(wb3)
            for ch in range(NCH):
                acc = pem.tile([TT, CHW], F32, tag="emacc", name=f"emacc{ch}")
                for k in range(4):
                    nc.tensor.matmul(
                        acc[:], wout_sb[k][:],
                        hhist["f"][:, (k * TOKq + ch * CHW):(k * TOKq + (ch + 1) * CHW)],
                        start=(k == 0), stop=False)
                for k in range(4):
                    nc.tensor.matmul(
                        acc[:], wout_sb[4 + k][:],
                        hhist["b"][:, (k * TOKq + ch * CHW):(k * TOKq + (ch + 1) * CHW)],
                        start=False, stop=(k == 3))
                nc.vector.tensor_scalar_add(
                    em_sb[:, ch * CHW:(ch + 1) * CHW], acc[:], bout_sb[:, :1])

        # ---- Phase E: gold score ----
        ppg = es.enter_context(tc.tile_pool(name="ppg", bufs=1))
        gold_emit_tok = ppg.tile([1, TOKq], F32, tag="g_emit")
        gold_trans_tok = ppg.tile([1, TOKq], F32, tag="g_trans")
        with tc.tile_pool(name="gold", bufs=3) as gld, \
             tc.tile_pool(name="pgold", bufs=1, space="PSUM") as pgd:
            for ch in range(NCH):
                sl = slice(ch * CHW, (ch + 1) * CHW)
                masks = {}
                for nm, tarr in (("next", tags_next), ("prev", tags_prev)):
                    trow = gld.tile([1, CHW], F32, tag=f"trow{nm}")
                    nc.sync.dma_start(trow[:], tarr[sl, :].rearrange("n 1 -> 1 n"))
                    tbc = pgd.tile([TT, CHW], F32, tag=f"tbc{nm}", name=f"tbc{nm}{ch}")
                    nc.tensor.matmul(tbc[:], ones_row[:1, :TT], trow[:], start=True, stop=True)
                    mask = gld.tile([TT, CHW], F32, tag=f"mask{nm}")
                    nc.vector.tensor_tensor(
                        out=mask[:], in0=tbc[:],
                        in1=iota_f[:, :1].to_broadcast([TT, CHW]),
                        op=ALU.is_equal)
                    masks[nm] = mask
                me = gld.tile([TT, CHW], F32, tag="me")
                nc.vector.tensor_mul(me[:], masks["next"][:], em_sb[:, sl])
                srow = pgd.tile([1, CHW], F32, tag="srow", name=f"srow{ch}")
                nc.tensor.matmul(srow[:], ones_col[:, :1], me[:], start=True, stop=True)
                nc.vector.tensor_copy(gold_emit_tok[:, sl], srow[:])
                ups = pgd.tile([TT, CHW], F32, tag="ups", name=f"ups{ch}")
                nc.tensor.matmul(ups[:], trans_sb[:], masks["prev"][:], start=True, stop=True)
                mu = gld.tile([TT, CHW], F32, tag="mu")
                nc.vector.tensor_mul(mu[:], masks["next"][:], ups[:])
                srow2 = pgd.tile([1, CHW], F32, tag="srow2", name=f"srow2{ch}")
                nc.tensor.matmul(srow2[:], ones_col[:, :1], mu[:], start=True, stop=True)
                nc.vector.tensor_copy(gold_trans_tok[:, sl], srow2[:])

        # ---- Phase F: CRF forward in exp space ----
        with tc.tile_pool(name="crf", bufs=2) as crf, \
             tc.tile_pool(name="crfst", bufs=1) as crfst, \
             tc.tile_pool(name="pcrf", bufs=1, space="PSUM") as pcf:
            e_mat = crfst.tile([TT, TT], F32, tag="e_mat")
            negc = crfst.tile([TT, 1], F32, tag="negc")
            nc.vector.memset(negc[:], -C_TRANS)
            nc.scalar.activation(e_mat[:], trans_sb[:], AF.Exp, bias=negc[:, :1])
            for ch in range(NCH):
                nc.scalar.activation(em_sb[:, ch * CHW:(ch + 1) * CHW],
                                     em_sb[:, ch * CHW:(ch + 1) * CHW], AF.Exp)
            # Step 0 analytically: alpha_1[n] = em_0[n] + NEG + log(w[n]) with
            # w[n] = exp(trans[n,0] - NEG) + sum_{p>0} exp(trans[n,p]); the NEG
            # start states are soft (-1e4), not -inf, so they contribute too.
            shift_col = crf.tile([TT, 1], F32, tag="shift_col")
            nc.vector.memset(shift_col[:], 0.0)
            nc.vector.memset(shift_col[:1, :], -NEG)
            ew = crf.tile([TT, TT], F32, tag="ew")
            nc.scalar.activation(ew[:], trans_sb[:], AF.Exp, bias=shift_col[:, :1])
            wps = pcf.tile([TT, 1], F32, tag="wps")
            nc.tensor.matmul(wps[:], ew[:], ones_col[:, :1], start=True, stop=True)
            w_col = crf.tile([TT, 1], F32, tag="w_col")
            nc.vector.tensor_copy(w_col[:], wps[:])
            a_sb = crfst.tile([TT, BC], F32, tag="a_sb")
            nc.vector.tensor_scalar_mul(a_sb[:], em_sb[:, 0:BC], w_col[:, :1])
            lzacc = crfst.tile([1, BC], F32, tag="lzacc")
            nc.vector.memset(lzacc[:], NEG)

            def renorm(tag):
                sps = pcf.tile([1, BC], F32, tag="sps", name=f"sps{tag}")
                nc.tensor.matmul(sps[:], ones_col[:, :1], a_sb[:], start=True, stop=True)
                lns = crf.tile([1, BC], F32, tag="lns", name=f"lns{tag}")
                nc.scalar.activation(lns[:], sps[:], AF.Ln)
                nc.vector.tensor_add(lzacc[:], lzacc[:], lns[:])
                rs_ = crf.tile([1, BC], F32, tag="rs", name=f"rs{tag}")
                nc.vector.reciprocal(rs_[:], sps[:])
                rbc = pcf.tile([TT, BC], F32, tag="rbc", name=f"rbc{tag}")
                nc.tensor.matmul(rbc[:], ones_row[:1, :TT], rs_[:], start=True, stop=True)
                nc.vector.tensor_mul(a_sb[:], a_sb[:], rbc[:])

            for t in range(1, Sq):
                aps = pcf.tile([TT, BC], F32, tag="aps", name=f"aps{t}")
                nc.tensor.matmul(aps[:], e_mat[:], a_sb[:], start=True, stop=True)
                nc.vector.tensor_mul(a_sb[:], aps[:], em_sb[:, t * BC:(t + 1) * BC])
                if t % RENORM == 0:
                    renorm(t)

            sfin = pcf.tile([1, BC], F32, tag="sfin")
            nc.tensor.matmul(sfin[:], ones_col[:, :1], a_sb[:], start=True, stop=True)
            lnf = crf.tile([1, BC], F32, tag="lnf")
            nc.scalar.activation(lnf[:], sfin[:], AF.Ln)
            logz = crf.tile([1, BC], F32, tag="logz")
            nc.vector.tensor_add(logz[:], lzacc[:], lnf[:])
            nc.vector.tensor_scalar_add(logz[:], logz[:], float(Sq - 1) * C_TRANS)
            gsum = crf.tile([1, BC], F32, tag="gsum")
            nc.vector.tensor_reduce(
                out=gsum[:],
                in_=gold_emit_tok[:1].rearrange("p (t b) -> p b t", b=BC),
                axis=mybir.AxisListType.X, op=ALU.add)
            gsum2 = crf.tile([1, BC], F32, tag="gsum2")
            nc.vector.tensor_reduce(
                out=gsum2[:],
                in_=gold_trans_tok[:1].rearrange("p (t b) -> p b t", b=BC),
                axis=mybir.AxisListType.X, op=ALU.add)
            nc.vector.tensor_add(gsum[:], gsum[:], gsum2[:])
            res = crf.tile([1, BC], F32, tag="res")
            nc.vector.tensor_sub(res[:], logz[:], gsum[:])
            nc.sync.dma_start(out_dram[:], res[:])

    nc.compile()
    return nc


_PROGRAM_CACHE = {}


def _get_program(nsteps=S):
    if nsteps not in _PROGRAM_CACHE:
        _PROGRAM_CACHE[nsteps] = build_program(nsteps)
    return _PROGRAM_CACHE[nsteps]


def make_in_maps(sentences, tags, emb, w_ih_f, w_hh_f, b_ih_f, b_hh_f,
                 w_ih_b, w_hh_b, b_ih_b, b_hh_b, w_out, b_out, transitions,
                 nsteps=S):
    """Host-side marshalling: shard batch, reorder to (t, b), transpose weights."""
    f32 = np.float32
    sentences = np.asarray(sentences)
    tags = np.asarray(tags)
    tags_ext = np.concatenate([np.zeros((B, 1), tags.dtype), tags], axis=1)
    shared = dict(
        emb=np.ascontiguousarray(np.asarray(emb, f32)),
        wihT_f=np.ascontiguousarray(np.asarray(w_ih_f, f32).T),
        wihT_b=np.ascontiguousarray(np.asarray(w_ih_b, f32).T),
        whhT_f=np.ascontiguousarray(np.asarray(w_hh_f, f32).T),
        whhT_b=np.ascontiguousarray(np.asarray(w_hh_b, f32).T),
        b_ih_f=np.asarray(b_ih_f, f32).reshape(G4, 1),
        b_hh_f=np.asarray(b_hh_f, f32).reshape(G4, 1),
        b_ih_b=np.asarray(b_ih_b, f32).reshape(G4, 1),
        b_hh_b=np.asarray(b_hh_b, f32).reshape(G4, 1),
        woutT=np.ascontiguousarray(np.asarray(w_out, f32).T),
        b_out=np.asarray(b_out, f32).reshape(TT, 1),
        transT=np.ascontiguousarray(np.asarray(transitions, f32).T),
    )
    in_maps = []
    for c in range(NCORES):
        bs = slice(c * BC, (c + 1) * BC)
        in_maps.append(dict(
            tok_idx=np.ascontiguousarray(
                sentences[bs, :nsteps].T.reshape(-1, 1).astype(np.int32)),
            tags_next=np.ascontiguousarray(
                tags[bs, :nsteps].T.reshape(-1, 1).astype(f32)),
            tags_prev=np.ascontiguousarray(
                tags_ext[bs, :nsteps].T.reshape(-1, 1).astype(f32)),
            **shared,
        ))
    return in_maps


def measure_exec_ns(inputs, repeats=30):
    """Device-resident repeat timing of the compiled program across 8 cores.

    Mirrors bass2jax.run_bass_via_pjrt's multi-core path but keeps inputs on
    device so per-call wall ~= dispatch + execution. Returns 25th-pct ns.
    """
    import time
    import jax
    from jax.sharding import Mesh, PartitionSpec
    from jax.experimental.shard_map import shard_map
    from concourse import bass2jax, mybir as _mb

    bass2jax.install_neuronx_cc_hook()
    nc = _get_program(S)
    in_maps = make_in_maps(
        inputs["sentences"], inputs["tags"], inputs["emb"],
        inputs["w_ih_f"], inputs["w_hh_f"], inputs["b_ih_f"], inputs["b_hh_f"],
        inputs["w_ih_b"], inputs["w_hh_b"], inputs["b_ih_b"], inputs["b_hh_b"],
        inputs["w_out"], inputs["b_out"], inputs["transitions"], nsteps=S)

    partition_name = (nc.partition_id_tensor.name
                      if nc.partition_id_tensor else None)
    in_names, out_names, out_avals, zero_outs = [], [], [], []
    for alloc in nc.m.functions[0].allocations:
        if not isinstance(alloc, _mb.MemoryLocationSet):
            continue
        name = alloc.memorylocations[0].name
        if alloc.kind == "ExternalInput":
            if name != partition_name:
                in_names.append(name)
        elif alloc.kind == "ExternalOutput":
            out_names.append(name)
            shape = tuple(alloc.tensor_shape)
            dtype = _mb.dt.np(alloc.dtype)
            out_avals.append(jax.core.ShapedArray(shape, dtype))
            zero_outs.append(np.zeros(shape, dtype))
    n_params = len(in_names)
    all_names = in_names + out_names
    if partition_name is not None:
        all_names = all_names + [partition_name]

    def _body(*args):
        operands = list(args)
        if partition_name is not None:
            operands.append(bass2jax.partition_id_tensor())
        outs = bass2jax._bass_exec_p.bind(
            *operands, out_avals=tuple(out_avals), in_names=tuple(all_names),
            out_names=tuple(out_names), lowering_input_output_aliases=(),
            sim_require_finite=True, sim_require_nnan=True, nc=nc)
        return tuple(outs)

    devices = jax.devices()[:NCORES]
    mesh = Mesh(np.asarray(devices), ("core",))
    n_outs = len(out_names)
    sharded = jax.jit(
        shard_map(_body, mesh=mesh,
                  in_specs=(PartitionSpec("core"),) * (n_params + n_outs),
                  out_specs=(PartitionSpec("core"),) * n_outs,
                  check_rep=False),
        keep_unused=True)
    concat_in = [
        np.concatenate([np.asarray(in_maps[c][nm]) for c in range(NCORES)], axis=0)
        for nm in in_names]
    concat_zeros = [np.zeros((NCORES * z.shape[0], *z.shape[1:]), z.dtype)
                    for z in zero_outs]
    dev_in = [jax.device_put(a) for a in concat_in]
    dev_zero = [jax.device_put(z) for z in concat_zeros]
    r = sharded(*dev_in, *dev_zero)
    jax.block_until_ready(r)
    samples = []
    for _ in range(repeats):
        t0 = time.perf_counter()
        r = sharded(*dev_in, *dev_zero)
        jax.block_until_ready(r)
        samples.append((time.perf_counter() - t0) * 1e9)
    samples.sort()
    return samples[len(samples) // 4]


def kernel(sentences, tags, lengths, emb, w_ih_f, w_hh_f, b_ih_f, b_hh_f,
           w_ih_b, w_hh_b, b_ih_b, b_hh_b, w_out, b_out, transitions):
    nc = _get_program(S)
    in_maps = make_in_maps(sentences, tags, emb, w_ih_f, w_hh_f, b_ih_f,
                           b_hh_f, w_ih_b, w_hh_b, b_ih_b, b_hh_b, w_out,
                           b_out, transitions, nsteps=S)
    res = run_bass_kernel_spmd(nc, in_maps, core_ids=list(range(NCORES)))
    parts = np.concatenate([r["out"].reshape(-1) for r in res.results])
    return np.float32(parts.mean())

